# revision 1
# baseline (speedup 1.0000x reference)
"""Trainium2 Bass kernel for nn_DotAlphaModule (sparse attention alpha).

Strategy (8 NeuronCores, SPMD):
  - Shard nodes N=8192 -> 1024/core; edges processed k-major (e = k*1024+n).
  - Full raw node table [8192, 9*128] bf16 replicated to every core's DRAM;
    neighbor rows fetched on-device with gpsimd.dma_gather (token-major).
  - sh(u) computed on device token-major; per-edge sh factors applied via
    diagonal matrices D_m = diag(sh_m) (built by one tensor_tensor against a
    masked-identity constant) feeding PE "transpose-matmuls" G_m^T @ D_m that
    accumulate the combined features S_l feature-major in PSUM.
  - Radial MLP, LayerNorms, fc and alpha-dot all run feature-major; LN stats
    via ones-matmuls on PE, rsqrt via DVE fast reciprocal + ACT sqrt.
  - All heavy matmuls in bf16 with f32 PSUM accumulation.
"""
import os
import sys
from contextlib import ExitStack

sys.path.insert(0, "/opt/trn_rl_repo")

import numpy as np
import ml_dtypes

import concourse.bass as bass
import concourse.tile as tile
import concourse.mybir as mybir
from concourse import bacc
from concourse.bass_utils import run_bass_kernel_spmd

BF16 = ml_dtypes.bfloat16

N, K = 8192, 32
NCORES = 8
NN = N // NCORES           # 1024 nodes per core
E = NN * K                 # 32768 edges per core
NCH = 9 * 128              # 1152 table row elements
NH, HD = 8, 32             # heads, head dim
CHUNK = 512                # edges per inner chunk
NCHUNK = E // CHUNK        # 64
EPS = 1e-5

C0 = 0.28209479177387814
C1 = 0.4886025119029199
C2 = 0.6307831305050401
S3 = 1.7320508075688772
NEG = 0.2

F32 = mybir.dt.float32
BF = mybir.dt.bfloat16
I16 = mybir.dt.int16
AF = mybir.ActivationFunctionType

# Native Silu runs on HW but is unimplemented in CoreSim; the fallback uses
# Sigmoid + an explicit multiply (identical math).
SILU_NATIVE = False


def _bap(ap, newap):
    return bass.AP(tensor=ap.tensor, offset=ap.offset, ap=newap)


def _build_nc(kmax=K):
    nc = bacc.Bacc("TRN2")
    # inputs
    tbl = nc.declare_dram_parameter("tbl", [N, NCH], BF, isOutput=False)
    selftbl = nc.declare_dram_parameter("selftbl", [NN, NCH], BF, isOutput=False)
    idxw = nc.declare_dram_parameter("idxw", [128, K * (NN // 16)], I16, isOutput=False)
    xT = nc.declare_dram_parameter("xT", [128, E], BF, isOutput=False)
    evp = nc.declare_dram_parameter("evp", [128, (E // 128) * 3], F32, isOutput=False)
    dwT = nc.declare_dram_parameter("dwT", [3, 128, 128], BF, isOutput=False)
    w0T = nc.declare_dram_parameter("w0T", [128, 64], BF, isOutput=False)
    w1T = nc.declare_dram_parameter("w1T", [64, 64], BF, isOutput=False)
    w2T = nc.declare_dram_parameter("w2T", [64, 768], BF, isOutput=False)
    fcT = nc.declare_dram_parameter("fcT", [6, 128, 256], BF, isOutput=False)
    aT = nc.declare_dram_parameter("aT", [2, 128, 12], BF, isOutput=False)
    mask8 = nc.declare_dram_parameter("mask8", [128, 8 * 128], BF, isOutput=False)
    ident = nc.declare_dram_parameter("ident", [128, 128], BF, isOutput=False)
    # f32 vector constants, packed [128, ncols]:
    # col 0: c0b (C0*dot_b), 1: b0, 2: g0, 3: bb0, 4: b1, 5: g1, 6: bb1,
    # cols 7-12: b2 blocks, 13-14: fcb halves, 15: gcol, 16: bcol, 17: abias(8)
    vcs = nc.declare_dram_parameter("vcs", [128, 18], F32, isOutput=False)
    on2 = nc.declare_dram_parameter("on2", [128, 2], F32, isOutput=False)
    bc2 = nc.declare_dram_parameter("bc2", [2, 128], F32, isOutput=False)
    onH = nc.declare_dram_parameter("onH", [128, 4], F32, isOutput=False)
    bcH = nc.declare_dram_parameter("bcH", [4, 128], F32, isOutput=False)
    outp = nc.declare_dram_parameter("out", [8, E], F32, isOutput=True)

    with tile.TileContext(nc) as tc, ExitStack() as ctx:
        cp = ctx.enter_context(tc.tile_pool(name="const", bufs=1))
        gp = ctx.enter_context(tc.tile_pool(name="gath", bufs=2))
        wk = ctx.enter_context(tc.tile_pool(name="work", bufs=2))
        wk3 = ctx.enter_context(tc.tile_pool(name="work3", bufs=3))
        psA = ctx.enter_context(tc.tile_pool(name="psA", bufs=3, space="PSUM"))
        psB = ctx.enter_context(tc.tile_pool(name="psB", bufs=2, space="PSUM"))
        psC = ctx.enter_context(tc.tile_pool(name="psC", bufs=3, space="PSUM"))

        def load_const(dram, shape, dt, nodma=False):
            t = cp.tile(shape, dt, tag=dram.name)
            if not nodma:
                nc.sync.dma_start(t[:], dram[:])
            return t

        ident_s = load_const(ident, [128, 128], BF)
        mask8_s = load_const(mask8, [128, 8, 128], BF)
        dwT_s = load_const(dwT, [128, 3, 128], BF, nodma=True)
        w0T_s = load_const(w0T, [128, 64], BF)
        w1T_s = load_const(w1T, [64, 64], BF)
        w2T_s = load_const(w2T, [64, 768], BF)
        fcT_s = load_const(fcT, [128, 6, 256], BF, nodma=True)
        aT_s = load_const(aT, [128, 2, 12], BF, nodma=True)
        vcs_s = load_const(vcs, [128, 18], F32)
        on2_s = load_const(on2, [128, 2], F32)
        bc2_s = load_const(bc2, [2, 128], F32)
        onH_s = load_const(onH, [128, 4], F32)
        bcH_s = load_const(bcH, [4, 128], F32)
        idx_s = load_const(idxw, [128, K * (NN // 16)], I16)
        evp_s = load_const(evp, [128, (E // 128) * 3], F32)

        # fix dwT / fcT / aT loads: dram dims (a, b, c) -> sbuf tile [b?..]
        # dwT dram [3, 128, 128] (l, c, d): load per l into [128, 3, 128]
        for l in range(3):
            nc.sync.dma_start(dwT_s[:, l, :], dwT[l, :, :])
        for b in range(6):
            nc.sync.dma_start(fcT_s[:, b, :], fcT[b, :, :])
        for h in range(2):
            nc.sync.dma_start(aT_s[:, h, :], aT[h, :, :])

        selfG = cp.tile([128, 8, NCH], BF, tag="selfG")
        for j in range(8):
            nc.sync.dma_start(selfG[:, j, :], selftbl[j * 128:(j + 1) * 128, :])

        c0b = vcs_s[:, 0:1]
        b0c = vcs_s[:64, 1:2]
        g0c = vcs_s[:64, 2:3]
        bb0c = vcs_s[:64, 3:4]
        b1c = vcs_s[:64, 4:5]
        g1c = vcs_s[:64, 5:6]
        bb1c = vcs_s[:64, 6:7]
        gcol = vcs_s[:, 15:16]
        bcol = vcs_s[:, 16:17]


        # ---- precompute Y0self [128d, 1024n] = C0*(W0 @ selftbl_m0^T) + C0*b ----
        s0s = cp.tile([128, 8, 128], BF, tag="s0s")
        for j in range(8):
            ps = psC.tile([128, 128], F32, tag="small")
            nc.tensor.matmul(ps[:], selfG[:, j, 0:128], ident_s[:], start=True, stop=True)
            nc.vector.tensor_copy(out=s0s[:, j, :], in_=ps[:])
        y0self = cp.tile([128, 1024], BF, tag="y0self")
        for h in range(2):
            ps = psA.tile([128, 512], F32, tag="mm")
            nc.tensor.matmul(ps[:], dwT_s[:, 0, :],
                             s0s[:].rearrange("p j c -> p (j c)")[:, h * 512:(h + 1) * 512],
                             start=True, stop=True)
            nc.scalar.add(y0self[:, h * 512:(h + 1) * 512], ps[:], add=c0b)

        M_OF_L = {1: [1, 2, 3], 2: [4, 5, 6, 7, 8]}

        for k in range(kmax):
            G = gp.tile([128, 8, NCH], BF, tag="G")
            nc.gpsimd.dma_gather(G[:], tbl[:], idx_s[:, k * 64:(k + 1) * 64],
                                 NN, NN, NCH)
            for half in range(2):
                ch = k * 2 + half          # chunk id
                col0 = ch * CHUNK          # global edge col
                tv = ch * (CHUNK // 128) * 3   # evp col offset (4 tiles * 3)

                # ---------------- sh [128, 4, 9] ----------------
                sh = wk3.tile([128, 4, 9], F32, tag="sh")
                shw = wk3.tile([128, 4, 4], F32, tag="shw")  # xx, zz, yy, n2
                evs = _bap(evp_s[:, tv:tv + 12], [evp_s.ap[0], [3, 4], [1, 3]])
                sq = wk3.tile([128, 4, 3], F32, tag="sq")
                nc.vector.tensor_mul(sq[:], evs, evs)
                n2 = shw[:, :, 3]
                nc.vector.tensor_reduce(n2, sq[:], mybir.AxisListType.X, mybir.AluOpType.add)
                nc.vector.tensor_scalar_add(n2, n2, 1e-20)
                rn2 = wk3.tile([128, 4], F32, tag="rn2")
                nc.vector.reciprocal_approx_fast(rn2[:], n2)
                nc.scalar.sqrt(rn2[:], rn2[:])          # 1/norm
                for t in range(4):
                    nc.vector.tensor_scalar_mul(sh[:, t, 1:4],
                                                evp_s[:, tv + 3 * t:tv + 3 * t + 3],
                                                rn2[:, t:t + 1])
                ux, uy, uz = sh[:, :, 1], sh[:, :, 2], sh[:, :, 3]
                nc.vector.tensor_mul(sh[:, :, 4], ux, uz)
                nc.vector.tensor_mul(sh[:, :, 5], ux, uy)
                nc.vector.tensor_mul(sh[:, :, 7], uy, uz)
                nc.vector.tensor_mul(shw[:, :, 0], ux, ux)
                nc.vector.tensor_mul(shw[:, :, 1], uz, uz)
                nc.vector.tensor_mul(shw[:, :, 2], uy, uy)
                axz = wk3.tile([128, 4], F32, tag="axz")
                nc.vector.tensor_add(axz[:], shw[:, :, 0], shw[:, :, 1])
                nc.vector.scalar_tensor_tensor(out=sh[:, :, 6], in0=axz[:], scalar=-0.5,
                                               in1=shw[:, :, 2],
                                               op0=mybir.AluOpType.mult,
                                               op1=mybir.AluOpType.add)
                nc.vector.tensor_sub(sh[:, :, 8], shw[:, :, 1], shw[:, :, 0])

                # ---------------- D matrices per tile ----------------
                Ds = []
                for t in range(4):
                    D = wk.tile([128, 8, 128], BF, tag=f"D{t}")
                    eng = nc.vector if t % 2 == 0 else nc.gpsimd
                    for mi in range(8):
                        eng.tensor_scalar_mul(D[:, mi, :], mask8_s[:, mi, :],
                                              sh[:, t, 1 + mi:2 + mi])
                    Ds.append(D)

                # ---------------- combine S blocks ----------------
                # blocks: 0=self0(precomputed) 1=neigh0 2=self1 3=neigh1 4=self2 5=neigh2
                s_sb = {}
                cpeng = [nc.scalar, nc.vector]
                bi = 0
                for blk, (src, l) in {1: ("n", 0), 2: ("s", 1), 3: ("n", 1),
                                      4: ("s", 2), 5: ("n", 2)}.items():
                    ps = psA.tile([128, 512], F32, tag="mm")
                    for t in range(4):
                        j = half * 4 + t
                        lhs_base = G if src == "n" else selfG
                        oap = ps[:, t * 128:(t + 1) * 128]
                        if l == 0:
                            nc.tensor.matmul(oap, lhs_base[:, j, 0:128], ident_s[:],
                                             start=True, stop=True)
                        else:
                            ms = M_OF_L[l]
                            for i, m in enumerate(ms):
                                nc.tensor.matmul(oap, lhs_base[:, j, m * 128:(m + 1) * 128],
                                                 Ds[t][:, m - 1, :],
                                                 start=(i == 0), stop=(i == len(ms) - 1))
                    sb = wk.tile([128, 512], BF, tag=f"ssb{blk}")
                    eng = cpeng[bi % 2]; bi += 1
                    if eng is nc.scalar:
                        nc.scalar.copy(sb[:], ps[:])
                    else:
                        eng.tensor_copy(out=sb[:], in_=ps[:])
                    s_sb[blk] = sb

                # ---------------- radial MLP ----------------
                xt = wk.tile([128, 512], BF, tag="xt")
                nc.sync.dma_start(xt[:], xT[:, col0:col0 + CHUNK])
                p0 = psC.tile([64, 512], F32, tag="small")
                nc.tensor.matmul(p0[:], w0T_s[:], xt[:], start=True, stop=True)

                def ln_block(pin, bcolv, gcolv, bbcolv):
                    stk = wk.tile([128, 512], F32, tag="stk")
                    nc.scalar.add(stk[:64, :], pin[:], add=bcolv)
                    nc.scalar.activation(out=stk[64:128, :], in_=pin[:], func=AF.Square,
                                         bias=bcolv, scale=1.0)
                    stp = psC.tile([2, 512], F32, tag="small")
                    nc.tensor.matmul(stp[:], on2_s[:], stk[:], start=True, stop=True)
                    sts = wk.tile([2, 512], F32, tag="stsrad")
                    nc.vector.tensor_copy(out=sts[:], in_=stp[:])
                    bcp = psB.tile([128, 512], F32, tag="bc")
                    nc.tensor.matmul(bcp[:], bc2_s[:], sts[:], start=True, stop=True)
                    mu = bcp[0:64, :]
                    s2 = bcp[64:128, :]
                    musq = wk.tile([64, 512], F32, tag="musq")
                    nc.scalar.square(musq[:], mu)
                    nc.vector.scalar_tensor_tensor(out=musq[:], in0=s2, scalar=EPS,
                                                   in1=musq[:],
                                                   op0=mybir.AluOpType.add,
                                                   op1=mybir.AluOpType.subtract)
                    nc.vector.reciprocal_approx_fast(musq[:], musq[:])
                    nc.scalar.sqrt(musq[:], musq[:])      # rsig [64, 512]
                    nc.vector.tensor_sub(stk[:64, :], stk[:64, :], mu)
                    t2 = wk.tile([64, 512], F32, tag="t2r")
                    nc.vector.tensor_mul(t2[:], stk[:64, :], musq[:])
                    ho = wk.tile([64, 512], BF, tag="ho")
                    if SILU_NATIVE:
                        nc.scalar.activation(out=ho[:], in_=t2[:], func=AF.Silu,
                                             bias=bbcolv, scale=gcolv)
                    else:
                        sg = wk.tile([64, 512], F32, tag="sg")
                        nc.scalar.activation(out=sg[:], in_=t2[:], func=AF.Sigmoid,
                                             bias=bbcolv, scale=gcolv)
                        ym = wk.tile([64, 512], F32, tag="ym")
                        nc.scalar.activation(out=ym[:], in_=t2[:], func=AF.Identity,
                                             bias=bbcolv, scale=gcolv)
                        nc.vector.tensor_mul(ho[:], ym[:], sg[:])
                    return ho

                h0 = ln_block(p0, b0c, g0c, bb0c)
                p1 = psC.tile([64, 512], F32, tag="small")
                nc.tensor.matmul(p1[:], w1T_s[:], h0[:], start=True, stop=True)
                h1 = ln_block(p1, b1c, g1c, bb1c)

                m0 = wk.tile([128, 6, 512], BF, tag="m0")
                for b in range(6):
                    pm = psA.tile([128, 512], F32, tag="mm")
                    nc.tensor.matmul(pm[:], w2T_s[:, b * 128:(b + 1) * 128], h1[:],
                                     start=True, stop=True)
                    b2c = vcs_s[:, 7 + b:8 + b]
                    if b % 2 == 0:
                        nc.scalar.add(m0[:, b, :], pm[:], add=b2c)
                    else:
                        nc.vector.tensor_scalar_add(m0[:, b, :], pm[:], b2c)

                # ---------------- x0 * m0 ----------------
                x0m = wk.tile([128, 6, 512], BF, tag="x0m")
                nc.vector.tensor_mul(x0m[:, 0, :], y0self[:, half * 512:(half + 1) * 512],
                                     m0[:, 0, :])
                for blk, (src, l) in {1: ("n", 0), 2: ("s", 1), 3: ("n", 1),
                                      4: ("s", 2), 5: ("n", 2)}.items():
                    yp = psA.tile([128, 512], F32, tag="mm")
                    nc.tensor.matmul(yp[:], dwT_s[:, l, :], s_sb[blk][:],
                                     start=True, stop=True)
                    if blk == 1:
                        nc.vector.scalar_tensor_tensor(out=x0m[:, 1, :], in0=yp[:],
                                                       scalar=c0b, in1=m0[:, 1, :],
                                                       op0=mybir.AluOpType.add,
                                                       op1=mybir.AluOpType.mult)
                    else:
                        nc.vector.tensor_mul(x0m[:, blk, :], yp[:], m0[:, blk, :])

                # ---------------- fc + final LN + alpha ----------------
                apsL = []
                for h2 in range(2):
                    zp = psA.tile([128, 512], F32, tag="mm")
                    for b in range(6):
                        nc.tensor.matmul(zp[:], fcT_s[:, b, h2 * 128:(h2 + 1) * 128],
                                         x0m[:, b, :], start=(b == 0), stop=(b == 5))
                    fcbc = vcs_s[:, 13 + h2:14 + h2]
                    zc = wk.tile([128, 512], F32, tag="zc")
                    nc.scalar.add(zc[:], zp[:], add=fcbc)
                    zsq = wk.tile([128, 512], F32, tag="zsq")
                    nc.vector.tensor_mul(zsq[:], zc[:], zc[:])
                    stpA = psC.tile([4, 512], F32, tag="small")
                    nc.tensor.matmul(stpA[:], onH_s[:], zc[:], start=True, stop=True)
                    stpB = psC.tile([4, 512], F32, tag="small")
                    nc.tensor.matmul(stpB[:], onH_s[:], zsq[:], start=True, stop=True)
                    stsA = wk.tile([4, 512], F32, tag="stsHa")
                    nc.vector.tensor_copy(out=stsA[:], in_=stpA[:])
                    stsB = wk.tile([4, 512], F32, tag="stsHb")
                    nc.vector.tensor_copy(out=stsB[:], in_=stpB[:])
                    musq = wk.tile([4, 512], F32, tag="musqH")
                    nc.vector.tensor_mul(musq[:], stsA[:], stsA[:])
                    nc.vector.scalar_tensor_tensor(out=musq[:], in0=stsB[:], scalar=EPS,
                                                   in1=musq[:],
                                                   op0=mybir.AluOpType.add,
                                                   op1=mybir.AluOpType.subtract)
                    nc.vector.reciprocal_approx_fast(musq[:], musq[:])
                    rsigB = wk.tile([4, 512], F32, tag="rsigH")
                    nc.scalar.sqrt(rsigB[:], musq[:])
                    mbp = psB.tile([128, 512], F32, tag="bc")
                    nc.tensor.matmul(mbp[:], bcH_s[:], stsA[:], start=True, stop=True)
                    rbp = psB.tile([128, 512], F32, tag="bc")
                    nc.tensor.matmul(rbp[:], bcH_s[:], rsigB[:], start=True, stop=True)
                    nc.vector.tensor_sub(zc[:], zc[:], mbp[:])
                    t2 = wk.tile([128, 512], BF, tag="t2H")
                    nc.vector.tensor_mul(t2[:], zc[:], rbp[:])
                    aps = psC.tile([4, 512], F32, tag="small")
                    if SILU_NATIVE:
                        sil = wk.tile([128, 512], BF, tag="silH")
                        nc.scalar.activation(out=sil[:], in_=t2[:], func=AF.Silu,
                                             bias=bcol, scale=gcol)
                        nc.tensor.matmul(aps[:], aT_s[:, h2, 0:4], t2[:], start=True, stop=False)
                        nc.tensor.matmul(aps[:], aT_s[:, h2, 4:8], sil[:], start=False, stop=True)
                    else:
                        sg = wk.tile([128, 512], BF, tag="sgH")
                        nc.scalar.activation(out=sg[:], in_=t2[:], func=AF.Sigmoid,
                                             bias=bcol, scale=gcol)
                        q = wk.tile([128, 512], BF, tag="qH")
                        nc.vector.tensor_mul(q[:], t2[:], sg[:])
                        nc.tensor.matmul(aps[:], aT_s[:, h2, 0:4], t2[:], start=True, stop=False)
                        nc.tensor.matmul(aps[:], aT_s[:, h2, 4:8], q[:], start=False, stop=False)
                        nc.tensor.matmul(aps[:], aT_s[:, h2, 8:12], sg[:], start=False, stop=True)
                    apsL.append(aps)

                for h2 in range(2):
                    asb = wk.tile([4, 512], F32, tag="asb")
                    ab = vcs_s[0:4, 17:18] if h2 == 0 else vcs_s[32:36, 17:18]
                    nc.scalar.add(asb[:], apsL[h2][:], add=ab)
                    nc.sync.dma_start(outp[h2 * 4:(h2 + 1) * 4, col0:col0 + CHUNK], asb[:])

    nc.compile()
    return nc


_NC = None


def _get_nc():
    global _NC
    if _NC is None:
        _NC = _build_nc()
    return _NC


def _host_prep(x_edge, node_irreps_input, edge_vec, f_sparse_idx_node,
               dot_w, dot_b, rad_w0, rad_b0, rad_w1, rad_b1, rad_w2, rad_b2,
               rad_g0, rad_bb0, rad_g1, rad_bb1, fc_w, fc_b, ln_g, ln_b, alpha_dot):
    f32 = np.float32
    tbl = np.ascontiguousarray(node_irreps_input.reshape(N, NCH)).astype(BF16)

    dwTn = np.zeros((3, 128, 128), f32)
    for l, s in enumerate([C0, C1, C2]):
        dwTn[l] = dot_w[l].T * s
    dwTn = dwTn.astype(BF16)

    w0Tn = rad_w0.T.astype(BF16)
    w1Tn = rad_w1.T.astype(BF16)
    w2Tn = rad_w2.T.astype(BF16)
    fcTn = np.ascontiguousarray(fc_w.T.reshape(6, 128, 256)).astype(BF16)

    aTn = np.zeros((2, 128, 12), f32)
    for hf in range(2):
        for hd in range(128):
            h_loc, dd = hd // 32, hd % 32
            a = alpha_dot[4 * hf + h_loc, dd]
            aTn[hf, hd, h_loc] = NEG * a * ln_g[dd]
            if SILU_NATIVE:
                aTn[hf, hd, 4 + h_loc] = (1 - NEG) * a
            else:
                aTn[hf, hd, 4 + h_loc] = (1 - NEG) * a * ln_g[dd]
                aTn[hf, hd, 8 + h_loc] = (1 - NEG) * a * ln_b[dd]
    aTn = aTn.astype(BF16)

    mask8n = np.zeros((128, 8 * 128), f32)
    diagv = [1.0, 1.0, 1.0, S3, S3, 1.0, S3, 0.5 * S3]  # m=1..8
    for mi in range(8):
        for p in range(128):
            mask8n[p, mi * 128 + p] = diagv[mi]
    mask8n = mask8n.astype(BF16)

    identn = np.eye(128, dtype=f32).astype(BF16)

    vcsn = np.zeros((128, 18), f32)
    vcsn[:, 0] = C0 * dot_b
    vcsn[:64, 1] = rad_b0
    vcsn[:64, 2] = rad_g0
    vcsn[:64, 3] = rad_bb0
    vcsn[:64, 4] = rad_b1
    vcsn[:64, 5] = rad_g1
    vcsn[:64, 6] = rad_bb1
    for b in range(6):
        vcsn[:, 7 + b] = rad_b2[b * 128:(b + 1) * 128]
    for h2 in range(2):
        vcsn[:, 13 + h2] = fc_b[h2 * 128:(h2 + 1) * 128]
    vcsn[:, 15] = np.tile(ln_g, 4)
    vcsn[:, 16] = np.tile(ln_b, 4)
    ab = NEG * (alpha_dot @ ln_b)
    vcsn[0:4, 17] = ab[0:4]
    vcsn[32:36, 17] = ab[4:8]

    on2n = np.zeros((128, 2), f32)
    on2n[:64, 0] = 1.0 / 64
    on2n[64:, 1] = 1.0 / 64
    bc2n = np.zeros((2, 128), f32)
    bc2n[0, :64] = 1.0
    bc2n[1, 64:] = 1.0
    onHn = np.zeros((128, 4), f32)
    for h in range(4):
        onHn[h * 32:(h + 1) * 32, h] = 1.0 / 32
    bcHn = np.zeros((4, 128), f32)
    for c in range(128):
        bcHn[c // 32, c] = 1.0
    shared = dict(tbl=tbl, dwT=dwTn, w0T=w0Tn, w1T=w1Tn, w2T=w2Tn, fcT=fcTn,
                  aT=aTn, mask8=mask8n, ident=identn, vcs=vcsn, on2=on2n,
                  bc2=bc2n, onH=onHn, bcH=bcHn)

    in_maps = []
    for c in range(NCORES):
        n0 = c * NN
        sl = slice(n0, n0 + NN)
        xc = x_edge[sl].astype(BF16)                     # [NN, K, 128]
        xTn = np.ascontiguousarray(np.transpose(xc, (2, 1, 0)).reshape(128, E))
        ev = edge_vec[sl].astype(f32)                    # [NN, K, 3]
        evkm = np.transpose(ev, (1, 0, 2)).reshape(E, 3)  # k-major [E, 3]
        evpn = np.ascontiguousarray(
            np.transpose(evkm.reshape(E // 128, 128, 3), (1, 0, 2)).reshape(128, (E // 128) * 3))
        idx = f_sparse_idx_node[sl].astype(np.int64).T.reshape(K, NN)  # k-major
        idxwn = np.zeros((128, K * (NN // 16)), np.int16)
        w = idx.reshape(K, NN // 16, 16).transpose(0, 2, 1)  # [K, 16, 64]
        for rep in range(8):
            idxwn[rep * 16:(rep + 1) * 16, :] = w.transpose(1, 0, 2).reshape(16, K * (NN // 16))
        selftbln = tbl[sl]
        m = dict(shared)
        m.update(xT=xTn, evp=evpn, idxw=idxwn, selftbl=selftbln)
        in_maps.append(m)
    return in_maps


def _assemble(results):
    full = np.zeros((N, K, NH), np.float32)
    for c in range(NCORES):
        o = results[c]["out"]                    # [8, E]
        full[c * NN:(c + 1) * NN] = np.transpose(o.reshape(NH, K, NN), (2, 1, 0))
    return full


def kernel(**inputs):
    nc = _get_nc()
    in_maps = _host_prep(**inputs)
    res = run_bass_kernel_spmd(nc, in_maps, core_ids=list(range(NCORES)))
    return _assemble(res.results)


if __name__ == "__main__":
    # quick single-core CoreSim correctness check on a reduced problem is not
    # practical (shapes hardcoded); use test.py against the reference instead.
    pass



# revision 17
# speedup vs baseline: 3.1751x; 3.1751x over previous
"""Trainium2 Bass kernel for nn_DotAlphaModule (sparse attention alpha).

Strategy (8 NeuronCores, SPMD):
  - Shard nodes N=8192 -> 1024/core; edges processed k-major (e = k*1024+n).
  - Full raw node table [8192, 9*128] bf16 replicated to every core's DRAM;
    neighbor rows fetched on-device with gpsimd.dma_gather (token-major).
  - sh(u) computed on device token-major; per-edge sh factors applied via
    diagonal matrices D_m = diag(sh_m) (built by one tensor_tensor against a
    masked-identity constant) feeding PE "transpose-matmuls" G_m^T @ D_m that
    accumulate the combined features S_l feature-major in PSUM.
  - Radial MLP, LayerNorms, fc and alpha-dot all run feature-major; LN stats
    via ones-matmuls on PE, rsqrt via DVE fast reciprocal + ACT sqrt.
  - All heavy matmuls in bf16 with f32 PSUM accumulation.
"""
import os
import sys
from contextlib import ExitStack

sys.path.insert(0, "/opt/trn_rl_repo")

import numpy as np
import ml_dtypes

import concourse.bass as bass
import concourse.tile as tile
import concourse.mybir as mybir
from concourse import bacc
from concourse.bass_utils import run_bass_kernel_spmd

BF16 = ml_dtypes.bfloat16

N, K = 8192, 32
NCORES = 8
NN = N // NCORES           # 1024 nodes per core
E = NN * K                 # 32768 edges per core
NCH = 9 * 128              # 1152 table row elements
NH, HD = 8, 32             # heads, head dim
CHUNK = 512                # edges per inner chunk
NCHUNK = E // CHUNK        # 64
EPS = 1e-5

C0 = 0.28209479177387814
C1 = 0.4886025119029199
C2 = 0.6307831305050401
S3 = 1.7320508075688772
NEG = 0.2

F32 = mybir.dt.float32
BF = mybir.dt.bfloat16
I16 = mybir.dt.int16
AF = mybir.ActivationFunctionType

# Native Silu runs on HW (silu_and_others ACT table); this path executes via
# neuronxcc+HW only (no CoreSim gate), so use it: saves 2 ACT + 1 DVE op per
# ln_block and 1 matmul + 1 DVE op per final half.
SILU_NATIVE = True


def _bap(ap, newap):
    return bass.AP(tensor=ap.tensor, offset=ap.offset, ap=newap)


def _build_nc(kmax=K):
    nc = bacc.Bacc("TRN2")
    # inputs
    # tbl/selftbl hold the HOST-PROJECTED node table Yt[j, m, d]:
    # per-l dot_w projection, C_l and diagv scales, and C0*dot_b (m=0)
    # all folded in, so the on-device diag matmuls produce y directly.
    tbl = nc.declare_dram_parameter("tbl", [N, NCH], BF, isOutput=False)
    selftbl = nc.declare_dram_parameter("selftbl", [NN, NCH], BF, isOutput=False)
    idxw = nc.declare_dram_parameter("idxw", [128, K * (NN // 16)], I16, isOutput=False)
    xT = nc.declare_dram_parameter("xT", [128, E], BF, isOutput=False)
    evp = nc.declare_dram_parameter("evp", [128, (E // 128) * 3], F32, isOutput=False)
    w0T = nc.declare_dram_parameter("w0T", [128, 64], BF, isOutput=False)
    w1T = nc.declare_dram_parameter("w1T", [64, 64], BF, isOutput=False)
    w2T = nc.declare_dram_parameter("w2T", [64, 768], BF, isOutput=False)
    fcT = nc.declare_dram_parameter("fcT", [6, 128, 256], BF, isOutput=False)
    aT = nc.declare_dram_parameter("aT", [2, 128, 12], BF, isOutput=False)
    mask8 = nc.declare_dram_parameter("mask8", [128, 8 * 128], BF, isOutput=False)
    ident = nc.declare_dram_parameter("ident", [128, 128], BF, isOutput=False)
    # f32 vector constants, packed [128, ncols]:
    # col 0: c0b (C0*dot_b), 1: b0, 2: g0, 3: bb0, 4: b1, 5: g1, 6: bb1,
    # cols 7-12: b2 blocks, 13-14: fcb halves, 15: gcol, 16: bcol, 17: abias(8)
    vcs = nc.declare_dram_parameter("vcs", [128, 18], F32, isOutput=False)
    on2 = nc.declare_dram_parameter("on2", [128, 2], F32, isOutput=False)
    bc2 = nc.declare_dram_parameter("bc2", [2, 128], F32, isOutput=False)
    onH = nc.declare_dram_parameter("onH", [128, 4], F32, isOutput=False)
    bcH = nc.declare_dram_parameter("bcH", [4, 128], F32, isOutput=False)
    outp = nc.declare_dram_parameter("out", [8, E], F32, isOutput=True)

    with tile.TileContext(nc) as tc, ExitStack() as ctx:
        cp = ctx.enter_context(tc.tile_pool(name="const", bufs=1))
        gp = ctx.enter_context(tc.tile_pool(name="gath", bufs=2))
        wk = ctx.enter_context(tc.tile_pool(name="work", bufs=2))
        wk3 = ctx.enter_context(tc.tile_pool(name="work3", bufs=3))
        psA = ctx.enter_context(tc.tile_pool(name="psA", bufs=3, space="PSUM"))
        psB = ctx.enter_context(tc.tile_pool(name="psB", bufs=2, space="PSUM"))
        psC = ctx.enter_context(tc.tile_pool(name="psC", bufs=3, space="PSUM"))

        def load_const(dram, shape, dt, nodma=False):
            t = cp.tile(shape, dt, tag=dram.name)
            if not nodma:
                nc.sync.dma_start(t[:], dram[:])
            return t

        ident_s = load_const(ident, [128, 128], BF)
        mask8_s = load_const(mask8, [128, 8, 128], BF)
        w0T_s = load_const(w0T, [128, 64], BF)
        w1T_s = load_const(w1T, [64, 64], BF)
        w2T_s = load_const(w2T, [64, 768], BF)
        fcT_s = load_const(fcT, [128, 6, 256], BF, nodma=True)
        aT_s = load_const(aT, [128, 2, 12], BF, nodma=True)
        vcs_s = load_const(vcs, [128, 18], F32)
        on2_s = load_const(on2, [128, 2], F32)
        bc2_s = load_const(bc2, [2, 128], F32)
        onH_s = load_const(onH, [128, 4], F32)
        bcH_s = load_const(bcH, [4, 128], F32)
        idx_s = load_const(idxw, [128, K * (NN // 16)], I16)
        evp_s = load_const(evp, [128, (E // 128) * 3], F32)

        for b in range(6):
            nc.sync.dma_start(fcT_s[:, b, :], fcT[b, :, :])
        for h in range(2):
            nc.sync.dma_start(aT_s[:, h, :], aT[h, :, :])

        selfG = cp.tile([128, 8, NCH], BF, tag="selfG")
        for j in range(8):
            nc.sync.dma_start(selfG[:, j, :], selftbl[j * 128:(j + 1) * 128, :])

        c0b = vcs_s[:, 0:1]
        b0c = vcs_s[:64, 1:2]
        g0c = vcs_s[:64, 2:3]
        bb0c = vcs_s[:64, 3:4]
        b1c = vcs_s[:64, 4:5]
        g1c = vcs_s[:64, 5:6]
        bb1c = vcs_s[:64, 6:7]
        gcol = vcs_s[:, 15:16]
        bcol = vcs_s[:, 16:17]


        # ---- y0self [128d, 1024n] = transpose of projected m0 block (bias
        # and C0 already folded into the table on the host) ----
        y0self = cp.tile([128, 1024], BF, tag="y0self")
        for j in range(8):
            ps = psC.tile([128, 128], F32, tag="small")
            nc.tensor.matmul(ps[:], selfG[:, j, 0:128], ident_s[:], start=True, stop=True)
            nc.vector.tensor_copy(out=y0self[:, j * 128:(j + 1) * 128], in_=ps[:])

        M_OF_L = {1: [1, 2, 3], 2: [4, 5, 6, 7, 8]}

        for k in range(kmax):
            G = gp.tile([128, 8, NCH], BF, tag="G")
            nc.gpsimd.dma_gather(G[:], tbl[:], idx_s[:, k * 64:(k + 1) * 64],
                                 NN, NN, NCH)

            # ---- radial MLP for BOTH halves first, staged so the two
            # halves' sqrt ops are adjacent and silu ops are adjacent in the
            # ACT stream (halves activation-table reloads) ----
            m0k = wk.tile([128, 6, 1024], BF, tag="m0k")
            p0L = []
            for half in range(2):
                col0 = (k * 2 + half) * CHUNK
                xt = wk.tile([128, 512], BF, tag=f"xt{half}")
                nc.sync.dma_start(xt[:], xT[:, col0:col0 + CHUNK])
                p0 = psC.tile([64, 512], F32, tag="small")
                nc.tensor.matmul(p0[:], w0T_s[:], xt[:], start=True, stop=True)
                p0L.append(p0)

            def ln_pre(pin, bcolv, tagp):
                stk = wk.tile([128, 512], F32, tag=f"stk{tagp}")
                nc.scalar.add(stk[:64, :], pin[:], add=bcolv)
                nc.scalar.activation(out=stk[64:128, :], in_=pin[:], func=AF.Square,
                                     bias=bcolv, scale=1.0)
                stp = psC.tile([2, 512], F32, tag="small")
                nc.tensor.matmul(stp[:], on2_s[:], stk[:], start=True, stop=True)
                sts = wk.tile([2, 512], F32, tag="stsrad")
                nc.vector.tensor_copy(out=sts[:], in_=stp[:])
                bcp = psB.tile([128, 512], F32, tag="bc")
                nc.tensor.matmul(bcp[:], bc2_s[:], sts[:], start=True, stop=True)
                mu = bcp[0:64, :]
                s2 = bcp[64:128, :]
                musq = wk.tile([64, 512], F32, tag=f"musq{tagp}")
                nc.scalar.square(musq[:], mu)
                nc.vector.scalar_tensor_tensor(out=musq[:], in0=s2, scalar=EPS,
                                               in1=musq[:],
                                               op0=mybir.AluOpType.add,
                                               op1=mybir.AluOpType.subtract)
                nc.vector.reciprocal_approx_fast(musq[:], musq[:])
                nc.vector.tensor_sub(stk[:64, :], stk[:64, :], mu)
                return stk, musq

            def ln_sqrt(musq, tagp):
                r = wk.tile([64, 512], F32, tag=f"rs{tagp}")
                nc.scalar.sqrt(r[:], musq[:])
                return r

            def ln_post(stk, rsig, gcolv, bbcolv, tagp):
                t2 = wk.tile([64, 512], F32, tag=f"t2r{tagp}")
                nc.vector.tensor_mul(t2[:], stk[:64, :], rsig[:])
                ho = wk.tile([64, 512], BF, tag=f"ho{tagp}")
                nc.scalar.activation(out=ho[:], in_=t2[:], func=AF.Silu,
                                     bias=bbcolv, scale=gcolv)
                return ho

            preL = [ln_pre(p0L[h], b0c, f"a{h}") for h in range(2)]
            rsL = [ln_sqrt(preL[h][1], f"a{h}") for h in range(2)]
            h0L = [ln_post(preL[h][0], rsL[h], g0c, bb0c, f"a{h}") for h in range(2)]
            p1L = []
            for h in range(2):
                p1 = psC.tile([64, 512], F32, tag="small")
                nc.tensor.matmul(p1[:], w1T_s[:], h0L[h][:], start=True, stop=True)
                p1L.append(p1)
            preL = [ln_pre(p1L[h], b1c, f"b{h}") for h in range(2)]
            rsL = [ln_sqrt(preL[h][1], f"b{h}") for h in range(2)]
            h1L = [ln_post(preL[h][0], rsL[h], g1c, bb1c, f"b{h}") for h in range(2)]
            for half in range(2):
                for b in range(6):
                    pm = psA.tile([128, 512], F32, tag="mm")
                    nc.tensor.matmul(pm[:], w2T_s[:, b * 128:(b + 1) * 128],
                                     h1L[half][:], start=True, stop=True)
                    b2c = vcs_s[:, 7 + b:8 + b]
                    dst = m0k[:, b, half * 512:(half + 1) * 512]
                    if b % 2 == 0:
                        nc.scalar.add(dst, pm[:], add=b2c)
                    else:
                        nc.vector.tensor_scalar_add(dst, pm[:], b2c)

            for half in range(2):
                ch = k * 2 + half          # chunk id
                col0 = ch * CHUNK          # global edge col
                tv = ch * (CHUNK // 128) * 3   # evp col offset (4 tiles * 3)

                # ---------------- sh [128, 4, 9] ----------------
                sh = wk3.tile([128, 4, 9], F32, tag="sh")
                shw = wk3.tile([128, 4, 4], F32, tag="shw")  # xx, zz, yy, n2
                evs = _bap(evp_s[:, tv:tv + 12], [evp_s.ap[0], [3, 4], [1, 3]])
                sq = wk3.tile([128, 4, 3], F32, tag="sq")
                nc.vector.tensor_mul(sq[:], evs, evs)
                n2 = shw[:, :, 3]
                nc.vector.tensor_reduce(n2, sq[:], mybir.AxisListType.X, mybir.AluOpType.add)
                nc.vector.tensor_scalar_add(n2, n2, 1e-20)
                rn2 = wk3.tile([128, 4], F32, tag="rn2")
                nc.vector.reciprocal_approx_fast(rn2[:], n2)
                nc.scalar.sqrt(rn2[:], rn2[:])          # 1/norm
                for t in range(4):
                    nc.vector.tensor_scalar_mul(sh[:, t, 1:4],
                                                evp_s[:, tv + 3 * t:tv + 3 * t + 3],
                                                rn2[:, t:t + 1])
                ux, uy, uz = sh[:, :, 1], sh[:, :, 2], sh[:, :, 3]
                nc.vector.tensor_mul(sh[:, :, 4], ux, uz)
                nc.vector.tensor_mul(sh[:, :, 5], ux, uy)
                nc.vector.tensor_mul(sh[:, :, 7], uy, uz)
                nc.vector.tensor_mul(shw[:, :, 0], ux, ux)
                nc.vector.tensor_mul(shw[:, :, 1], uz, uz)
                nc.vector.tensor_mul(shw[:, :, 2], uy, uy)
                axz = wk3.tile([128, 4], F32, tag="axz")
                nc.vector.tensor_add(axz[:], shw[:, :, 0], shw[:, :, 1])
                nc.vector.scalar_tensor_tensor(out=sh[:, :, 6], in0=axz[:], scalar=-0.5,
                                               in1=shw[:, :, 2],
                                               op0=mybir.AluOpType.mult,
                                               op1=mybir.AluOpType.add)
                nc.vector.tensor_sub(sh[:, :, 8], shw[:, :, 1], shw[:, :, 0])

                # ---------------- D matrices: one stride-0 broadcast op/tile ----
                shb = wk3.tile([128, 4, 8], BF, tag="shb")
                nc.vector.tensor_copy(out=shb[:], in_=sh[:, :, 1:9])
                Ds = []
                for t in range(4):
                    D = wk.tile([128, 8, 128], BF, tag=f"D{t}")
                    sl = shb[:, t, 0:8]
                    shbc = _bap(sl, [sl.ap[0], [1, 8], [0, 128]])
                    nc.vector.tensor_mul(D[:], mask8_s[:], shbc)
                    Ds.append(D)

                # ---------------- y blocks (diag MMs) fused with x0 * m0 ----
                # blocks: 0=self0(precomputed) 1=neigh0 2=self1 3=neigh1 4=self2 5=neigh2
                x0m = wk.tile([128, 6, 512], BF, tag="x0m")
                nc.vector.tensor_mul(x0m[:, 0, :], y0self[:, half * 512:(half + 1) * 512],
                                     m0k[:, 0, half * 512:(half + 1) * 512])
                for blk, (src, l) in {1: ("n", 0), 2: ("s", 1), 3: ("n", 1),
                                      4: ("s", 2), 5: ("n", 2)}.items():
                    ps = psA.tile([128, 512], F32, tag="mm")
                    for t in range(4):
                        j = half * 4 + t
                        lhs_base = G if src == "n" else selfG
                        oap = ps[:, t * 128:(t + 1) * 128]
                        if l == 0:
                            nc.tensor.matmul(oap, lhs_base[:, j, 0:128], ident_s[:],
                                             start=True, stop=True)
                        else:
                            ms = M_OF_L[l]
                            for i, m in enumerate(ms):
                                nc.tensor.matmul(oap, lhs_base[:, j, m * 128:(m + 1) * 128],
                                                 Ds[t][:, m - 1, :],
                                                 start=(i == 0), stop=(i == len(ms) - 1))
                    nc.vector.tensor_mul(x0m[:, blk, :], ps[:], m0k[:, blk, half * 512:(half + 1) * 512])

                # ---------------- fc + final LN + alpha (staged halves:
                # sqrts adjacent, silus adjacent -> fewer table reloads) ----
                zcL, stsAL, musqL, rsigL, t2L, silL = [], [], [], [], [], []
                for h2 in range(2):
                    zp = psA.tile([128, 512], F32, tag="mm")
                    for b in range(6):
                        nc.tensor.matmul(zp[:], fcT_s[:, b, h2 * 128:(h2 + 1) * 128],
                                         x0m[:, b, :], start=(b == 0), stop=(b == 5))
                    fcbc = vcs_s[:, 13 + h2:14 + h2]
                    zc = wk.tile([128, 512], F32, tag=f"zc{h2}")
                    nc.scalar.add(zc[:], zp[:], add=fcbc)
                    zsq = wk.tile([128, 512], F32, tag="zsq")
                    nc.vector.tensor_mul(zsq[:], zc[:], zc[:])
                    stpA = psC.tile([4, 512], F32, tag="small")
                    nc.tensor.matmul(stpA[:], onH_s[:], zc[:], start=True, stop=True)
                    stpB = psC.tile([4, 512], F32, tag="small")
                    nc.tensor.matmul(stpB[:], onH_s[:], zsq[:], start=True, stop=True)
                    stsA = wk.tile([4, 512], F32, tag=f"stsHa{h2}")
                    nc.vector.tensor_copy(out=stsA[:], in_=stpA[:])
                    stsB = wk.tile([4, 512], F32, tag="stsHb")
                    nc.vector.tensor_copy(out=stsB[:], in_=stpB[:])
                    musq = wk.tile([4, 512], F32, tag=f"musqH{h2}")
                    nc.vector.tensor_mul(musq[:], stsA[:], stsA[:])
                    nc.vector.scalar_tensor_tensor(out=musq[:], in0=stsB[:], scalar=EPS,
                                                   in1=musq[:],
                                                   op0=mybir.AluOpType.add,
                                                   op1=mybir.AluOpType.subtract)
                    nc.vector.reciprocal_approx_fast(musq[:], musq[:])
                    zcL.append(zc); stsAL.append(stsA); musqL.append(musq)
                for h2 in range(2):
                    rsigB = wk.tile([4, 512], F32, tag=f"rsigH{h2}")
                    nc.scalar.sqrt(rsigB[:], musqL[h2][:])
                    rsigL.append(rsigB)
                for h2 in range(2):
                    mbp = psB.tile([128, 512], F32, tag="bc")
                    nc.tensor.matmul(mbp[:], bcH_s[:], stsAL[h2][:], start=True, stop=True)
                    rbp = psB.tile([128, 512], F32, tag="bc")
                    nc.tensor.matmul(rbp[:], bcH_s[:], rsigL[h2][:], start=True, stop=True)
                    zc = zcL[h2]
                    nc.vector.tensor_sub(zc[:], zc[:], mbp[:])
                    t2 = wk.tile([128, 512], BF, tag=f"t2H{h2}")
                    nc.vector.tensor_mul(t2[:], zc[:], rbp[:])
                    t2L.append(t2)
                for h2 in range(2):
                    sil = wk.tile([128, 512], BF, tag=f"silH{h2}")
                    nc.scalar.activation(out=sil[:], in_=t2L[h2][:], func=AF.Silu,
                                         bias=bcol, scale=gcol)
                    silL.append(sil)
                for h2 in range(2):
                    aps = psC.tile([4, 512], F32, tag="small")
                    nc.tensor.matmul(aps[:], aT_s[:, h2, 0:4], t2L[h2][:],
                                     start=True, stop=False)
                    nc.tensor.matmul(aps[:], aT_s[:, h2, 4:8], silL[h2][:],
                                     start=False, stop=True)
                    asb = wk.tile([4, 512], F32, tag="asb")
                    ab = vcs_s[0:4, 17:18] if h2 == 0 else vcs_s[32:36, 17:18]
                    nc.scalar.add(asb[:], aps[:], add=ab)
                    nc.sync.dma_start(outp[h2 * 4:(h2 + 1) * 4, col0:col0 + CHUNK], asb[:])

    nc.compile()
    return nc


_NC = None


def _get_nc():
    global _NC
    if _NC is None:
        _NC = _build_nc()
    return _NC


def _host_prep(x_edge, node_irreps_input, edge_vec, f_sparse_idx_node,
               dot_w, dot_b, rad_w0, rad_b0, rad_w1, rad_b1, rad_w2, rad_b2,
               rad_g0, rad_bb0, rad_g1, rad_bb1, fc_w, fc_b, ln_g, ln_b, alpha_dot):
    f32 = np.float32
    # host-projected table: Yt[j, m, d] with dot_w, C_l, diagv, bias folded
    raw = np.asarray(node_irreps_input, f32)            # [N, 9, 128]
    diagv = [1.0, 1.0, 1.0, S3, S3, 1.0, S3, 0.5 * S3]  # m=1..8
    Yt = np.empty((N, 9, 128), f32)
    for l, s in enumerate([C0, C1, C2]):
        sl = slice(l * l, (l + 1) * (l + 1))
        Yt[:, sl] = (raw[:, sl].reshape(-1, 128) @ (dot_w[l].T * s)).reshape(
            N, 2 * l + 1, 128)
    Yt[:, 0] += C0 * np.asarray(dot_b, f32)
    for mi in range(8):
        Yt[:, mi + 1] *= diagv[mi]
    tbl = np.ascontiguousarray(Yt.reshape(N, NCH)).astype(BF16)

    w0Tn = rad_w0.T.astype(BF16)
    w1Tn = rad_w1.T.astype(BF16)
    w2Tn = rad_w2.T.astype(BF16)
    fcTn = np.ascontiguousarray(fc_w.T.reshape(6, 128, 256)).astype(BF16)

    aTn = np.zeros((2, 128, 12), f32)
    for hf in range(2):
        for hd in range(128):
            h_loc, dd = hd // 32, hd % 32
            a = alpha_dot[4 * hf + h_loc, dd]
            aTn[hf, hd, h_loc] = NEG * a * ln_g[dd]
            if SILU_NATIVE:
                aTn[hf, hd, 4 + h_loc] = (1 - NEG) * a
            else:
                aTn[hf, hd, 4 + h_loc] = (1 - NEG) * a * ln_g[dd]
                aTn[hf, hd, 8 + h_loc] = (1 - NEG) * a * ln_b[dd]
    aTn = aTn.astype(BF16)

    mask8n = np.zeros((128, 8 * 128), f32)
    for mi in range(8):
        for p in range(128):
            mask8n[p, mi * 128 + p] = 1.0
    mask8n = mask8n.astype(BF16)

    identn = np.eye(128, dtype=f32).astype(BF16)

    vcsn = np.zeros((128, 18), f32)
    vcsn[:, 0] = C0 * dot_b
    vcsn[:64, 1] = rad_b0
    vcsn[:64, 2] = rad_g0
    vcsn[:64, 3] = rad_bb0
    vcsn[:64, 4] = rad_b1
    vcsn[:64, 5] = rad_g1
    vcsn[:64, 6] = rad_bb1
    for b in range(6):
        vcsn[:, 7 + b] = rad_b2[b * 128:(b + 1) * 128]
    for h2 in range(2):
        vcsn[:, 13 + h2] = fc_b[h2 * 128:(h2 + 1) * 128]
    vcsn[:, 15] = np.tile(ln_g, 4)
    vcsn[:, 16] = np.tile(ln_b, 4)
    ab = NEG * (alpha_dot @ ln_b)
    vcsn[0:4, 17] = ab[0:4]
    vcsn[32:36, 17] = ab[4:8]

    on2n = np.zeros((128, 2), f32)
    on2n[:64, 0] = 1.0 / 64
    on2n[64:, 1] = 1.0 / 64
    bc2n = np.zeros((2, 128), f32)
    bc2n[0, :64] = 1.0
    bc2n[1, 64:] = 1.0
    onHn = np.zeros((128, 4), f32)
    for h in range(4):
        onHn[h * 32:(h + 1) * 32, h] = 1.0 / 32
    bcHn = np.zeros((4, 128), f32)
    for c in range(128):
        bcHn[c // 32, c] = 1.0
    shared = dict(tbl=tbl, w0T=w0Tn, w1T=w1Tn, w2T=w2Tn, fcT=fcTn,
                  aT=aTn, mask8=mask8n, ident=identn, vcs=vcsn, on2=on2n,
                  bc2=bc2n, onH=onHn, bcH=bcHn)

    in_maps = []
    for c in range(NCORES):
        n0 = c * NN
        sl = slice(n0, n0 + NN)
        xc = x_edge[sl].astype(BF16)                     # [NN, K, 128]
        xTn = np.ascontiguousarray(np.transpose(xc, (2, 1, 0)).reshape(128, E))
        ev = edge_vec[sl].astype(f32)                    # [NN, K, 3]
        evkm = np.transpose(ev, (1, 0, 2)).reshape(E, 3)  # k-major [E, 3]
        evpn = np.ascontiguousarray(
            np.transpose(evkm.reshape(E // 128, 128, 3), (1, 0, 2)).reshape(128, (E // 128) * 3))
        idx = f_sparse_idx_node[sl].astype(np.int64).T.reshape(K, NN)  # k-major
        idxwn = np.zeros((128, K * (NN // 16)), np.int16)
        w = idx.reshape(K, NN // 16, 16).transpose(0, 2, 1)  # [K, 16, 64]
        for rep in range(8):
            idxwn[rep * 16:(rep + 1) * 16, :] = w.transpose(1, 0, 2).reshape(16, K * (NN // 16))
        selftbln = tbl[sl]
        m = dict(shared)
        m.update(xT=xTn, evp=evpn, idxw=idxwn, selftbl=selftbln)
        in_maps.append(m)
    return in_maps


def _assemble(results):
    full = np.zeros((N, K, NH), np.float32)
    for c in range(NCORES):
        o = results[c]["out"]                    # [8, E]
        full[c * NN:(c + 1) * NN] = np.transpose(o.reshape(NH, K, NN), (2, 1, 0))
    return full


def kernel(**inputs):
    nc = _get_nc()
    in_maps = _host_prep(**inputs)
    res = run_bass_kernel_spmd(nc, in_maps, core_ids=list(range(NCORES)))
    return _assemble(res.results)


if __name__ == "__main__":
    # quick single-core CoreSim correctness check on a reduced problem is not
    # practical (shapes hardcoded); use test.py against the reference instead.
    pass



# revision 19
# speedup vs baseline: 3.1999x; 1.0078x over previous
"""Trainium2 Bass kernel for nn_DotAlphaModule (sparse attention alpha).

Strategy (8 NeuronCores, SPMD):
  - Shard nodes N=8192 -> 1024/core; edges processed k-major (e = k*1024+n).
  - Full raw node table [8192, 9*128] bf16 replicated to every core's DRAM;
    neighbor rows fetched on-device with gpsimd.dma_gather (token-major).
  - sh(u) computed on device token-major; per-edge sh factors applied via
    diagonal matrices D_m = diag(sh_m) (built by one tensor_tensor against a
    masked-identity constant) feeding PE "transpose-matmuls" G_m^T @ D_m that
    accumulate the combined features S_l feature-major in PSUM.
  - Radial MLP, LayerNorms, fc and alpha-dot all run feature-major; LN stats
    via ones-matmuls on PE, rsqrt via DVE fast reciprocal + ACT sqrt.
  - All heavy matmuls in bf16 with f32 PSUM accumulation.
"""
import os
import sys
from contextlib import ExitStack

sys.path.insert(0, "/opt/trn_rl_repo")

import numpy as np
import ml_dtypes

import concourse.bass as bass
import concourse.tile as tile
import concourse.mybir as mybir
from concourse import bacc
from concourse.bass_utils import run_bass_kernel_spmd

BF16 = ml_dtypes.bfloat16

N, K = 8192, 32
NCORES = 8
NN = N // NCORES           # 1024 nodes per core
E = NN * K                 # 32768 edges per core
NCH = 9 * 128              # 1152 table row elements
NH, HD = 8, 32             # heads, head dim
CHUNK = 512                # edges per inner chunk
NCHUNK = E // CHUNK        # 64
EPS = 1e-5

C0 = 0.28209479177387814
C1 = 0.4886025119029199
C2 = 0.6307831305050401
S3 = 1.7320508075688772
NEG = 0.2

F32 = mybir.dt.float32
BF = mybir.dt.bfloat16
I16 = mybir.dt.int16
AF = mybir.ActivationFunctionType

# Native Silu runs on HW (silu_and_others ACT table); this path executes via
# neuronxcc+HW only (no CoreSim gate), so use it: saves 2 ACT + 1 DVE op per
# ln_block and 1 matmul + 1 DVE op per final half.
SILU_NATIVE = True


def _bap(ap, newap):
    return bass.AP(tensor=ap.tensor, offset=ap.offset, ap=newap)


def _build_nc(kmax=K):
    nc = bacc.Bacc("TRN2")
    # inputs
    # tbl/selftbl hold the HOST-PROJECTED node table Yt[j, m, d]:
    # per-l dot_w projection, C_l and diagv scales, and C0*dot_b (m=0)
    # all folded in, so the on-device diag matmuls produce y directly.
    tbl = nc.declare_dram_parameter("tbl", [N, NCH], BF, isOutput=False)
    selftbl = nc.declare_dram_parameter("selftbl", [NN, NCH], BF, isOutput=False)
    idxw = nc.declare_dram_parameter("idxw", [128, K * (NN // 16)], I16, isOutput=False)
    xT = nc.declare_dram_parameter("xT", [128, E], BF, isOutput=False)
    evp = nc.declare_dram_parameter("evp", [128, (E // 128) * 3], F32, isOutput=False)
    w0T = nc.declare_dram_parameter("w0T", [128, 64], BF, isOutput=False)
    w1T = nc.declare_dram_parameter("w1T", [64, 64], BF, isOutput=False)
    w2T = nc.declare_dram_parameter("w2T", [64, 768], BF, isOutput=False)
    fcT = nc.declare_dram_parameter("fcT", [6, 128, 256], BF, isOutput=False)
    aT = nc.declare_dram_parameter("aT", [2, 128, 12], BF, isOutput=False)
    mask8 = nc.declare_dram_parameter("mask8", [128, 8 * 128], BF, isOutput=False)
    ident = nc.declare_dram_parameter("ident", [128, 128], BF, isOutput=False)
    # f32 vector constants, packed [128, ncols]:
    # col 0: c0b (C0*dot_b), 1: b0, 2: g0, 3: bb0, 4: b1, 5: g1, 6: bb1,
    # cols 7-12: b2 blocks, 13-14: fcb halves, 15: gcol, 16: bcol, 17: abias(8)
    vcs = nc.declare_dram_parameter("vcs", [128, 18], F32, isOutput=False)
    on2 = nc.declare_dram_parameter("on2", [128, 2], F32, isOutput=False)
    bc2 = nc.declare_dram_parameter("bc2", [2, 128], F32, isOutput=False)
    onH = nc.declare_dram_parameter("onH", [128, 4], F32, isOutput=False)
    bcH = nc.declare_dram_parameter("bcH", [4, 128], F32, isOutput=False)
    outp = nc.declare_dram_parameter("out", [8, E], F32, isOutput=True)

    with tile.TileContext(nc) as tc, ExitStack() as ctx:
        cp = ctx.enter_context(tc.tile_pool(name="const", bufs=1))
        gp = ctx.enter_context(tc.tile_pool(name="gath", bufs=2))
        wk = ctx.enter_context(tc.tile_pool(name="work", bufs=2))
        wk3 = ctx.enter_context(tc.tile_pool(name="work3", bufs=3))
        psA = ctx.enter_context(tc.tile_pool(name="psA", bufs=3, space="PSUM"))
        psB = ctx.enter_context(tc.tile_pool(name="psB", bufs=2, space="PSUM"))
        psC = ctx.enter_context(tc.tile_pool(name="psC", bufs=3, space="PSUM"))

        def load_const(dram, shape, dt, nodma=False):
            t = cp.tile(shape, dt, tag=dram.name)
            if not nodma:
                nc.sync.dma_start(t[:], dram[:])
            return t

        ident_s = load_const(ident, [128, 128], BF)
        mask8_s = load_const(mask8, [128, 8, 128], BF)
        w0T_s = load_const(w0T, [128, 64], BF)
        w1T_s = load_const(w1T, [64, 64], BF)
        w2T_s = load_const(w2T, [64, 768], BF)
        fcT_s = load_const(fcT, [128, 6, 256], BF, nodma=True)
        aT_s = load_const(aT, [128, 2, 12], BF, nodma=True)
        vcs_s = load_const(vcs, [128, 18], F32)
        on2_s = load_const(on2, [128, 2], F32)
        bc2_s = load_const(bc2, [2, 128], F32)
        onH_s = load_const(onH, [128, 4], F32)
        bcH_s = load_const(bcH, [4, 128], F32)
        idx_s = load_const(idxw, [128, K * (NN // 16)], I16)
        evp_s = load_const(evp, [128, (E // 128) * 3], F32)

        for b in range(6):
            nc.sync.dma_start(fcT_s[:, b, :], fcT[b, :, :])
        for h in range(2):
            nc.sync.dma_start(aT_s[:, h, :], aT[h, :, :])

        selfG = cp.tile([128, 8, NCH], BF, tag="selfG")
        for j in range(8):
            nc.sync.dma_start(selfG[:, j, :], selftbl[j * 128:(j + 1) * 128, :])

        c0b = vcs_s[:, 0:1]
        b0c = vcs_s[:64, 1:2]
        g0c = vcs_s[:64, 2:3]
        bb0c = vcs_s[:64, 3:4]
        b1c = vcs_s[:64, 4:5]
        g1c = vcs_s[:64, 5:6]
        bb1c = vcs_s[:64, 6:7]
        gcol = vcs_s[:, 15:16]
        bcol = vcs_s[:, 16:17]


        # ---- y0self [128d, 1024n] = transpose of projected m0 block (bias
        # and C0 already folded into the table on the host) ----
        y0self = cp.tile([128, 1024], BF, tag="y0self")
        for j in range(8):
            ps = psC.tile([128, 128], F32, tag="small")
            nc.tensor.matmul(ps[:], selfG[:, j, 0:128], ident_s[:], start=True, stop=True)
            nc.vector.tensor_copy(out=y0self[:, j * 128:(j + 1) * 128], in_=ps[:])

        M_OF_L = {1: [1, 2, 3], 2: [4, 5, 6, 7, 8]}

        for k in range(kmax):
            G = gp.tile([128, 8, NCH], BF, tag="G")
            nc.gpsimd.dma_gather(G[:], tbl[:], idx_s[:, k * 64:(k + 1) * 64],
                                 NN, NN, NCH)

            # ---- radial MLP for BOTH halves first, staged so the two
            # halves' sqrt ops are adjacent and silu ops are adjacent in the
            # ACT stream (halves activation-table reloads) ----
            m0k = wk.tile([128, 6, 1024], BF, tag="m0k")
            p0L = []
            for half in range(2):
                col0 = (k * 2 + half) * CHUNK
                xt = wk.tile([128, 512], BF, tag=f"xt{half}")
                nc.sync.dma_start(xt[:], xT[:, col0:col0 + CHUNK])
                p0 = psC.tile([64, 512], F32, tag="small")
                nc.tensor.matmul(p0[:], w0T_s[:], xt[:], start=True, stop=True)
                p0L.append(p0)

            def ln_pre(pin, bcolv, tagp):
                stk = wk.tile([128, 512], F32, tag=f"stk{tagp}")
                nc.scalar.add(stk[:64, :], pin[:], add=bcolv)
                nc.scalar.activation(out=stk[64:128, :], in_=pin[:], func=AF.Square,
                                     bias=bcolv, scale=1.0)
                stp = psC.tile([2, 512], F32, tag="small")
                nc.tensor.matmul(stp[:], on2_s[:], stk[:], start=True, stop=True)
                sts = wk.tile([2, 512], F32, tag="stsrad")
                nc.vector.tensor_copy(out=sts[:], in_=stp[:])
                bcp = psB.tile([128, 512], F32, tag="bc")
                nc.tensor.matmul(bcp[:], bc2_s[:], sts[:], start=True, stop=True)
                mu = bcp[0:64, :]
                s2 = bcp[64:128, :]
                musq = wk.tile([64, 512], F32, tag=f"musq{tagp}")
                nc.scalar.square(musq[:], mu)
                nc.vector.scalar_tensor_tensor(out=musq[:], in0=s2, scalar=EPS,
                                               in1=musq[:],
                                               op0=mybir.AluOpType.add,
                                               op1=mybir.AluOpType.subtract)
                nc.vector.reciprocal_approx_fast(musq[:], musq[:])
                nc.vector.tensor_sub(stk[:64, :], stk[:64, :], mu)
                return stk, musq

            def ln_sqrt(musq, tagp):
                r = wk.tile([64, 512], F32, tag=f"rs{tagp}")
                nc.scalar.sqrt(r[:], musq[:])
                return r

            def ln_post(stk, rsig, gcolv, bbcolv, tagp):
                t2 = wk.tile([64, 512], F32, tag=f"t2r{tagp}")
                nc.vector.tensor_mul(t2[:], stk[:64, :], rsig[:])
                ho = wk.tile([64, 512], BF, tag=f"ho{tagp}")
                nc.scalar.activation(out=ho[:], in_=t2[:], func=AF.Silu,
                                     bias=bbcolv, scale=gcolv)
                return ho

            preL = [ln_pre(p0L[h], b0c, f"a{h}") for h in range(2)]
            rsL = [ln_sqrt(preL[h][1], f"a{h}") for h in range(2)]
            h0L = [ln_post(preL[h][0], rsL[h], g0c, bb0c, f"a{h}") for h in range(2)]
            p1L = []
            for h in range(2):
                p1 = psC.tile([64, 512], F32, tag="small")
                nc.tensor.matmul(p1[:], w1T_s[:], h0L[h][:], start=True, stop=True)
                p1L.append(p1)
            preL = [ln_pre(p1L[h], b1c, f"b{h}") for h in range(2)]
            rsL = [ln_sqrt(preL[h][1], f"b{h}") for h in range(2)]
            h1L = [ln_post(preL[h][0], rsL[h], g1c, bb1c, f"b{h}") for h in range(2)]
            for half in range(2):
                for b in range(6):
                    pm = psA.tile([128, 512], F32, tag="mm")
                    nc.tensor.matmul(pm[:], w2T_s[:, b * 128:(b + 1) * 128],
                                     h1L[half][:], start=True, stop=True)
                    b2c = vcs_s[:, 7 + b:8 + b]
                    dst = m0k[:, b, half * 512:(half + 1) * 512]
                    if b % 2 == 0:
                        nc.scalar.add(dst, pm[:], add=b2c)
                    else:
                        nc.vector.tensor_scalar_add(dst, pm[:], b2c)

            for half in range(2):
                ch = k * 2 + half          # chunk id
                col0 = ch * CHUNK          # global edge col
                tv = ch * (CHUNK // 128) * 3   # evp col offset (4 tiles * 3)

                # ---------------- sh [128, 4, 9] ----------------
                sh = wk3.tile([128, 4, 9], F32, tag="sh")
                shw = wk3.tile([128, 4, 4], F32, tag="shw")  # xx, zz, yy, n2
                evs = _bap(evp_s[:, tv:tv + 12], [evp_s.ap[0], [3, 4], [1, 3]])
                sq = wk3.tile([128, 4, 3], F32, tag="sq")
                nc.vector.tensor_mul(sq[:], evs, evs)
                n2 = shw[:, :, 3]
                nc.vector.tensor_reduce(n2, sq[:], mybir.AxisListType.X, mybir.AluOpType.add)
                nc.vector.tensor_scalar_add(n2, n2, 1e-20)
                rn2 = wk3.tile([128, 4], F32, tag="rn2")
                nc.vector.reciprocal_approx_fast(rn2[:], n2)
                nc.scalar.sqrt(rn2[:], rn2[:])          # 1/norm
                for t in range(4):
                    nc.vector.tensor_scalar_mul(sh[:, t, 1:4],
                                                evp_s[:, tv + 3 * t:tv + 3 * t + 3],
                                                rn2[:, t:t + 1])
                ux, uy, uz = sh[:, :, 1], sh[:, :, 2], sh[:, :, 3]
                nc.vector.tensor_mul(sh[:, :, 4], ux, uz)
                nc.vector.tensor_mul(sh[:, :, 5], ux, uy)
                nc.vector.tensor_mul(sh[:, :, 7], uy, uz)
                nc.vector.tensor_mul(shw[:, :, 0], ux, ux)
                nc.vector.tensor_mul(shw[:, :, 1], uz, uz)
                nc.vector.tensor_mul(shw[:, :, 2], uy, uy)
                axz = wk3.tile([128, 4], F32, tag="axz")
                nc.vector.tensor_add(axz[:], shw[:, :, 0], shw[:, :, 1])
                nc.vector.scalar_tensor_tensor(out=sh[:, :, 6], in0=axz[:], scalar=-0.5,
                                               in1=shw[:, :, 2],
                                               op0=mybir.AluOpType.mult,
                                               op1=mybir.AluOpType.add)
                nc.vector.tensor_sub(sh[:, :, 8], shw[:, :, 1], shw[:, :, 0])

                # ---------------- D matrices: one stride-0 broadcast op/tile ----
                shb = wk3.tile([128, 4, 8], BF, tag="shb")
                nc.vector.tensor_copy(out=shb[:], in_=sh[:, :, 1:9])
                Ds = []
                for t in range(4):
                    D = wk.tile([128, 8, 128], BF, tag=f"D{t}")
                    sl = shb[:, t, 0:8]
                    shbc = _bap(sl, [sl.ap[0], [1, 8], [0, 128]])
                    nc.vector.tensor_mul(D[:], mask8_s[:], shbc)
                    Ds.append(D)

                # ---------------- y blocks (diag MMs) fused with x0 * m0 ----
                # blocks: 0=self0(precomputed) 1=neigh0 2=self1 3=neigh1 4=self2 5=neigh2
                x0m = wk.tile([128, 6, 512], BF, tag="x0m")
                nc.vector.tensor_mul(x0m[:, 0, :], y0self[:, half * 512:(half + 1) * 512],
                                     m0k[:, 0, half * 512:(half + 1) * 512])
                for blk, (src, l) in {1: ("n", 0), 2: ("s", 1), 3: ("n", 1),
                                      4: ("s", 2), 5: ("n", 2)}.items():
                    ps = psA.tile([128, 512], F32, tag="mm")
                    for t in range(4):
                        j = half * 4 + t
                        lhs_base = G if src == "n" else selfG
                        oap = ps[:, t * 128:(t + 1) * 128]
                        if l == 0:
                            nc.tensor.matmul(oap, lhs_base[:, j, 0:128], ident_s[:],
                                             start=True, stop=True)
                        else:
                            ms = M_OF_L[l]
                            for i, m in enumerate(ms):
                                nc.tensor.matmul(oap, lhs_base[:, j, m * 128:(m + 1) * 128],
                                                 Ds[t][:, m - 1, :],
                                                 start=(i == 0), stop=(i == len(ms) - 1))
                    nc.vector.tensor_mul(x0m[:, blk, :], ps[:], m0k[:, blk, half * 512:(half + 1) * 512])

                # ---------------- fc + final LN + alpha (staged halves:
                # sqrts adjacent, silus adjacent -> fewer table reloads) ----
                zcL, stsAL, musqL, rsigL, t2L, silL = [], [], [], [], [], []
                for h2 in range(2):
                    zp = psA.tile([128, 512], F32, tag="mm")
                    for b in range(6):
                        nc.tensor.matmul(zp[:], fcT_s[:, b, h2 * 128:(h2 + 1) * 128],
                                         x0m[:, b, :], start=(b == 0), stop=(b == 5))
                    fcbc = vcs_s[:, 13 + h2:14 + h2]
                    zc = wk.tile([128, 512], F32, tag=f"zc{h2}")
                    nc.scalar.add(zc[:], zp[:], add=fcbc)
                    zsq = wk.tile([128, 512], F32, tag="zsq")
                    nc.vector.tensor_mul(zsq[:], zc[:], zc[:])
                    stpA = psC.tile([4, 512], F32, tag="small")
                    nc.tensor.matmul(stpA[:], onH_s[:], zc[:], start=True, stop=True)
                    stpB = psC.tile([4, 512], F32, tag="small")
                    nc.tensor.matmul(stpB[:], onH_s[:], zsq[:], start=True, stop=True)
                    stsA = wk.tile([4, 512], F32, tag=f"stsHa{h2}")
                    nc.vector.tensor_copy(out=stsA[:], in_=stpA[:])
                    stsB = wk.tile([4, 512], F32, tag="stsHb")
                    nc.vector.tensor_copy(out=stsB[:], in_=stpB[:])
                    musq = wk.tile([4, 512], F32, tag=f"musqH{h2}")
                    nc.vector.tensor_mul(musq[:], stsA[:], stsA[:])
                    nc.vector.scalar_tensor_tensor(out=musq[:], in0=stsB[:], scalar=EPS,
                                                   in1=musq[:],
                                                   op0=mybir.AluOpType.add,
                                                   op1=mybir.AluOpType.subtract)
                    nc.vector.reciprocal_approx_fast(musq[:], musq[:])
                    zcL.append(zc); stsAL.append(stsA); musqL.append(musq)
                for h2 in range(2):
                    rsigB = wk.tile([4, 512], F32, tag=f"rsigH{h2}")
                    nc.scalar.sqrt(rsigB[:], musqL[h2][:])
                    rsigL.append(rsigB)
                for h2 in range(2):
                    mbp = psB.tile([128, 512], F32, tag="bc")
                    nc.tensor.matmul(mbp[:], bcH_s[:], stsAL[h2][:], start=True, stop=True)
                    rbp = psB.tile([128, 512], F32, tag="bc")
                    nc.tensor.matmul(rbp[:], bcH_s[:], rsigL[h2][:], start=True, stop=True)
                    zc = zcL[h2]
                    nc.vector.tensor_sub(zc[:], zc[:], mbp[:])
                    t2 = wk.tile([128, 512], BF, tag=f"t2H{h2}")
                    nc.vector.tensor_mul(t2[:], zc[:], rbp[:])
                    t2L.append(t2)
                for h2 in range(2):
                    sil = wk.tile([128, 512], BF, tag=f"silH{h2}")
                    nc.scalar.activation(out=sil[:], in_=t2L[h2][:], func=AF.Silu,
                                         bias=bcol, scale=gcol)
                    silL.append(sil)
                for h2 in range(2):
                    aps = psC.tile([4, 512], F32, tag="small")
                    nc.tensor.matmul(aps[:], aT_s[:, h2, 0:4], t2L[h2][:],
                                     start=True, stop=False)
                    nc.tensor.matmul(aps[:], aT_s[:, h2, 4:8], silL[h2][:],
                                     start=False, stop=True)
                    asb = wk.tile([4, 512], F32, tag="asb")
                    ab = vcs_s[0:4, 17:18] if h2 == 0 else vcs_s[32:36, 17:18]
                    nc.scalar.add(asb[:], aps[:], add=ab)
                    nc.sync.dma_start(outp[h2 * 4:(h2 + 1) * 4, col0:col0 + CHUNK], asb[:])

    nc.compile()
    return nc


_NC = None


def _get_nc():
    global _NC
    if _NC is None:
        _NC = _build_nc()
    return _NC


def _host_prep(x_edge, node_irreps_input, edge_vec, f_sparse_idx_node,
               dot_w, dot_b, rad_w0, rad_b0, rad_w1, rad_b1, rad_w2, rad_b2,
               rad_g0, rad_bb0, rad_g1, rad_bb1, fc_w, fc_b, ln_g, ln_b, alpha_dot):
    f32 = np.float32
    # host-projected table: Yt[j, m, d] with dot_w, C_l, diagv, bias folded
    raw = np.asarray(node_irreps_input, f32)            # [N, 9, 128]
    diagv = [1.0, 1.0, 1.0, S3, S3, 1.0, S3, 0.5 * S3]  # m=1..8
    Yt = np.empty((N, 9, 128), f32)
    for l, s in enumerate([C0, C1, C2]):
        sl = slice(l * l, (l + 1) * (l + 1))
        Yt[:, sl] = (raw[:, sl].reshape(-1, 128) @ (dot_w[l].T * s)).reshape(
            N, 2 * l + 1, 128)
    Yt[:, 0] += C0 * np.asarray(dot_b, f32)
    for mi in range(8):
        Yt[:, mi + 1] *= diagv[mi]
    tbl = np.ascontiguousarray(Yt.reshape(N, NCH)).astype(BF16)

    w0Tn = rad_w0.T.astype(BF16)
    w1Tn = rad_w1.T.astype(BF16)
    w2Tn = rad_w2.T.astype(BF16)
    fcTn = np.ascontiguousarray(fc_w.T.reshape(6, 128, 256)).astype(BF16)

    aTn = np.zeros((2, 128, 12), f32)
    for hf in range(2):
        for hd in range(128):
            h_loc, dd = hd // 32, hd % 32
            a = alpha_dot[4 * hf + h_loc, dd]
            aTn[hf, hd, h_loc] = NEG * a * ln_g[dd]
            if SILU_NATIVE:
                aTn[hf, hd, 4 + h_loc] = (1 - NEG) * a
            else:
                aTn[hf, hd, 4 + h_loc] = (1 - NEG) * a * ln_g[dd]
                aTn[hf, hd, 8 + h_loc] = (1 - NEG) * a * ln_b[dd]
    aTn = aTn.astype(BF16)

    mask8n = np.zeros((128, 8 * 128), f32)
    for mi in range(8):
        for p in range(128):
            mask8n[p, mi * 128 + p] = 1.0
    mask8n = mask8n.astype(BF16)

    identn = np.eye(128, dtype=f32).astype(BF16)

    vcsn = np.zeros((128, 18), f32)
    vcsn[:, 0] = C0 * dot_b
    vcsn[:64, 1] = rad_b0
    vcsn[:64, 2] = rad_g0
    vcsn[:64, 3] = rad_bb0
    vcsn[:64, 4] = rad_b1
    vcsn[:64, 5] = rad_g1
    vcsn[:64, 6] = rad_bb1
    for b in range(6):
        vcsn[:, 7 + b] = rad_b2[b * 128:(b + 1) * 128]
    for h2 in range(2):
        vcsn[:, 13 + h2] = fc_b[h2 * 128:(h2 + 1) * 128]
    vcsn[:, 15] = np.tile(ln_g, 4)
    vcsn[:, 16] = np.tile(ln_b, 4)
    ab = NEG * (alpha_dot @ ln_b)
    vcsn[0:4, 17] = ab[0:4]
    vcsn[32:36, 17] = ab[4:8]

    on2n = np.zeros((128, 2), f32)
    on2n[:64, 0] = 1.0 / 64
    on2n[64:, 1] = 1.0 / 64
    bc2n = np.zeros((2, 128), f32)
    bc2n[0, :64] = 1.0
    bc2n[1, 64:] = 1.0
    onHn = np.zeros((128, 4), f32)
    for h in range(4):
        onHn[h * 32:(h + 1) * 32, h] = 1.0 / 32
    bcHn = np.zeros((4, 128), f32)
    for c in range(128):
        bcHn[c // 32, c] = 1.0
    shared = dict(tbl=tbl, w0T=w0Tn, w1T=w1Tn, w2T=w2Tn, fcT=fcTn,
                  aT=aTn, mask8=mask8n, ident=identn, vcs=vcsn, on2=on2n,
                  bc2=bc2n, onH=onHn, bcH=bcHn)

    in_maps = []
    for c in range(NCORES):
        n0 = c * NN
        sl = slice(n0, n0 + NN)
        xc = x_edge[sl].astype(BF16)                     # [NN, K, 128]
        xTn = np.ascontiguousarray(np.transpose(xc, (2, 1, 0)).reshape(128, E))
        ev = edge_vec[sl].astype(f32)                    # [NN, K, 3]
        evkm = np.transpose(ev, (1, 0, 2)).reshape(E, 3)  # k-major [E, 3]
        evpn = np.ascontiguousarray(
            np.transpose(evkm.reshape(E // 128, 128, 3), (1, 0, 2)).reshape(128, (E // 128) * 3))
        idx = f_sparse_idx_node[sl].astype(np.int64).T.reshape(K, NN)  # k-major
        idxwn = np.zeros((128, K * (NN // 16)), np.int16)
        w = idx.reshape(K, NN // 16, 16).transpose(0, 2, 1)  # [K, 16, 64]
        for rep in range(8):
            idxwn[rep * 16:(rep + 1) * 16, :] = w.transpose(1, 0, 2).reshape(16, K * (NN // 16))
        selftbln = tbl[sl]
        m = dict(shared)
        m.update(xT=xTn, evp=evpn, idxw=idxwn, selftbl=selftbln)
        in_maps.append(m)
    return in_maps


def _assemble(results):
    full = np.zeros((N, K, NH), np.float32)
    for c in range(NCORES):
        o = results[c]["out"]                    # [8, E]
        full[c * NN:(c + 1) * NN] = np.transpose(o.reshape(NH, K, NN), (2, 1, 0))
    return full


def kernel(**inputs):
    nc = _get_nc()
    in_maps = _host_prep(**inputs)
    res = run_bass_kernel_spmd(nc, in_maps, core_ids=list(range(NCORES)))
    return _assemble(res.results)


if __name__ == "__main__":
    # quick single-core CoreSim correctness check on a reduced problem is not
    # practical (shapes hardcoded); use test.py against the reference instead.
    pass

